# revision 16
# baseline (speedup 1.0000x reference)
"""Trainium2 Bass kernel for a 3-layer bidirectional GRU + dense sigmoid head.

Problem: B=256, T=512, D=256, H=128 (Keras reset_after=True, gate order z,r,h).
Sharding: data-parallel over batch, 32 examples per core on 8 NeuronCores.

Per-core design (gate-partition layout, everything [128(h-dim), cols]):
- Input projections (xp = x @ W + b) are computed as chunked GEMMs whose
  outputs land directly in PSUM banks; the sequential scan's recurrence
  matmuls (h @ U) then accumulate on top of the same PSUM columns, so no
  PSUM->SBUF staging of xp is ever needed.
- Per 16-step group, PSUM banks: [z_f | r_f | z_b | r_b] (xp+rec, sigmoid
  reads all 4 with one strided AP), [xph_f | xph_b] (xp only), and a small
  per-step scratch bank for rec_h.
- Forward and backward chains are interleaved (independent recurrences) so
  the PE/ACT/DVE pipeline always has work.
- matmuls run in float32r (relaxed fp32), accumulation in fp32 PSUM.
"""

from contextlib import ExitStack

import numpy as np

import concourse.bass as bass
from concourse import bacc
import concourse.mybir as mybir
import concourse.tile as tile
from concourse.bass_utils import run_bass_kernel_spmd

H = 128
D_IN = 256
N_CORES = 8
F32 = mybir.dt.float32
F32R = mybir.dt.float32r
AF = mybir.ActivationFunctionType


def _r(ap):
    return ap.bitcast(F32R)


def build_gru(nc, B, T, L, GRP, has_bias, has_bhh):
    """Emit the full GRU program into `nc`."""
    NG = T // GRP
    assert T % GRP == 0

    # packed weights: cols [w | u | wd | bias(row0) | bhh(rows0-1)]
    CW = L * 2 * 2 * 3 * H          # 4608
    CU = L * 2 * 3 * H              # 2304
    c_u = CW
    c_wd = CW + CU
    c_bias = c_wd + 2
    c_bhh = c_bias + CU
    c_ones = c_bhh + L * H
    c_ind2 = c_ones + GRP * B
    c_h0 = c_ind2 + 2 * B
    C = c_h0 + 2 * B
    x = nc.dram_tensor("x", [D_IN, T * B], F32R, kind="ExternalInput")
    wpack = nc.dram_tensor("wpack", [H, C], F32R, kind="ExternalInput")
    y = nc.dram_tensor("y", [1, B], F32, kind="ExternalOutput")

    with tile.TileContext(nc) as tc, ExitStack() as ctx:
        const = ctx.enter_context(tc.tile_pool(name="const", bufs=1))
        rhsp = ctx.enter_context(tc.tile_pool(name="rhsp", bufs=2))
        outp = ctx.enter_context(tc.tile_pool(name="outp", bufs=2))
        stepp = ctx.enter_context(tc.tile_pool(name="stepp", bufs=3))
        psum = ctx.enter_context(tc.tile_pool(name="psum", bufs=1,
                                              space="PSUM"))
        pscr = ctx.enter_context(tc.tile_pool(name="pscr", bufs=2,
                                              space="PSUM"))
        dramp = ctx.enter_context(tc.tile_pool(name="dramp", bufs=1,
                                               space="DRAM"))

        # inter-layer hidden-sequence buffers (Tile-tracked DRAM)
        seqs = []
        for p in "AB":
            sf = dramp.tile([H, T * B], F32R, name=f"seq{p}f", tag=f"seq{p}f")
            sb = dramp.tile([H, T * B], F32R, name=f"seq{p}b", tag=f"seq{p}b")
            seqs.append((sf, sb))

        # ---- preload all weights with a single contiguous DMA ----
        pk = const.tile([H, C], F32R)
        nc.sync.dma_start(out=pk, in_=wpack[:])

        def w_ap(l, d, k, gi):
            c = ((l * 2 + d) * 2 + k) * 3 * H + gi * H
            return pk[:, c:c + H]

        def u_ap(l, d, gi):
            c = c_u + (l * 2 + d) * 3 * H + gi * H
            return pk[:, c:c + H]

        def wd_ap(d):
            return pk[:, c_wd + d:c_wd + d + 1]

        def bias_ap(l, d, gi):
            c = c_bias + (l * 2 + d) * 3 * H + gi * H
            return pk[0:1, c:c + H]

        def bhh_ap(l):
            return pk[0:2, c_bhh + l * H:c_bhh + (l + 1) * H]

        h0_sb = pk[:, c_h0:c_h0 + 2 * B].rearrange("p (d b) -> p d b", d=2)
        ones_sb = pk[0:1, c_ones:c_ones + GRP * B]
        ind2_sb = pk[0:2, c_ind2:c_ind2 + 2 * B]

        prev_out = None  # previous group's outbuf (h carry within a layer)
        outbuf = None

        for l in range(L):
            for g in range(NG):
                t_hi = T - 1 - GRP * g  # top t of the bwd group's range
                # ---- rhs tiles (moving operand of the xp GEMM) ----
                rhs = {}
                for d, dn in ((0, "f"), (1, "b")):
                    for k in range(2):
                        rt = rhsp.tile([H, GRP, B], F32R, tag=f"rhs{dn}{k}",
                                       name=f"rhs_{dn}{k}_{l}_{g}")
                        if l == 0:
                            s_fb = x[:][k * H:(k + 1) * H, :]
                        else:
                            s_fb = seqs[(l - 1) % 2][k]  # k0=fwd, k1=bwd half
                        if d == 0:
                            src = s_fb.rearrange("p (t b) -> p t b", b=B)[
                                :, GRP * g:GRP * (g + 1), :]
                        else:
                            src = bass.AP(
                                tensor=s_fb.tensor,
                                offset=s_fb.offset + t_hi * B,
                                ap=[[T * B, H], [-B, GRP], [1, B]])
                        nc.sync.dma_start(out=rt, in_=src)
                        rhs[(d, k)] = rt

                # ---- PSUM banks ----
                zrb = psum.tile([H, 4, GRP, B], F32, tag="zrb",
                                name=f"zrb_{l}_{g}")
                xph = psum.tile([H, 2, GRP, B], F32, tag="xph",
                                name=f"xph_{l}_{g}")

                # ---- xp GEMM: accumulate x @ W (+ b) into the banks ----
                for d in (0, 1):
                    for gi in range(3):
                        out_ap = (zrb[:, 2 * d + gi, :, :] if gi < 2
                                  else xph[:, d, :, :])
                        for k in range(2):
                            nc.tensor.matmul(
                                out_ap,
                                _r(w_ap(l, d, k, gi)),
                                _r(rhs[(d, k)]),
                                start=(k == 0), stop=False,
                                skip_group_check=True)
                        if has_bias:
                            nc.tensor.matmul(
                                out_ap,
                                _r(bias_ap(l, d, gi)),
                                _r(ones_sb),
                                start=False, stop=False,
                                skip_group_check=True)

                outbuf = outp.tile([H, 2, GRP, B], F32R, tag="outbuf",
                                   name=f"outbuf_{l}_{g}")

                # ---- the sequential scan: GRP fwd+bwd step-pairs ----
                for tl in range(GRP):
                    if prev_out is None and tl == 0:
                        hprev = h0_sb  # [H, 2, B] zeros
                    elif tl == 0:
                        hprev = prev_out[:, :, GRP - 1, :]
                    else:
                        hprev = outbuf[:, :, tl - 1, :]

                    scratch = pscr.tile([H, 2, B], F32, tag="scratch",
                                        name=f"scr_{l}_{g}_{tl}")
                    for d in (0, 1):
                        hp_d = _r(hprev[:, d, :])
                        for gi in range(3):
                            out_ap = (zrb[:, 2 * d + gi, tl, :] if gi < 2
                                      else scratch[:, d, :])
                            # h-gate writes the fresh scratch bank: start
                            # clears has_written for the WHOLE bank, so only
                            # the first direction may set it.
                            nc.tensor.matmul(
                                out_ap,
                                _r(u_ap(l, d, gi)),
                                hp_d,
                                start=(gi == 2 and d == 0), stop=True,
                                skip_group_check=True)
                    if has_bhh:
                        nc.tensor.matmul(
                            scratch[:, :, :], _r(bhh_ap(l)),
                            _r(ind2_sb), start=False, stop=True,
                            skip_group_check=True)

                    # sigmoid over [z_f | r_f | z_b | r_b], one strided read
                    zrout = stepp.tile([H, 4, B], F32, tag="zrout",
                                       name=f"zrout_{l}_{g}_{tl}")
                    nc.scalar.activation(zrout, zrb[:, :, tl, :], AF.Sigmoid)
                    pstride = zrout.ap[0][0]
                    z_ap = bass.AP(tensor=zrout.tensor, offset=zrout.offset,
                                   ap=[[pstride, H], [2 * B, 2], [1, B]])
                    r_ap = bass.AP(tensor=zrout.tensor,
                                   offset=zrout.offset + B,
                                   ap=[[pstride, H], [2 * B, 2], [1, B]])

                    tt = stepp.tile([H, 2, B], F32, tag="tt",
                                    name=f"tt_{l}_{g}_{tl}")
                    nc.vector.tensor_mul(tt, scratch, r_ap)
                    arg = stepp.tile([H, 2, B], F32, tag="arg",
                                     name=f"arg_{l}_{g}_{tl}")
                    nc.vector.tensor_add(arg, tt, xph[:, :, tl, :])
                    hh = stepp.tile([H, 2, B], F32, tag="hh",
                                    name=f"hh_{l}_{g}_{tl}")
                    nc.scalar.activation(hh, arg, AF.Tanh)
                    dd = stepp.tile([H, 2, B], F32, tag="dd",
                                    name=f"dd_{l}_{g}_{tl}")
                    nc.vector.tensor_sub(dd, hprev, hh)
                    ee = stepp.tile([H, 2, B], F32, tag="ee",
                                    name=f"ee_{l}_{g}_{tl}")
                    nc.vector.tensor_mul(ee, z_ap, dd)
                    nc.vector.tensor_add(outbuf[:, :, tl, :], ee, hh)

                # ---- store the group's hidden states (layers 0..L-2) ----
                if l < L - 1:
                    sf, sb = seqs[l % 2]
                    nc.sync.dma_start(
                        out=sf.rearrange("p (t b) -> p t b", b=B)[
                            :, GRP * g:GRP * (g + 1), :],
                        in_=outbuf[:, 0, :, :])
                    nc.sync.dma_start(
                        out=bass.AP(tensor=sb.tensor,
                                    offset=sb.offset + t_hi * B,
                                    ap=[[T * B, H], [-B, GRP], [1, B]]),
                        in_=outbuf[:, 1, :, :])
                prev_out = outbuf
            prev_out = None  # h resets between layers

        # ---- dense head on the final states of the last group ----
        py = pscr.tile([1, B], F32, tag="scratch", name="py")
        nc.tensor.matmul(py, _r(wd_ap(0)),
                         _r(outbuf[:, 0, GRP - 1, :]),
                         start=True, stop=False, skip_group_check=True)
        nc.tensor.matmul(py, _r(wd_ap(1)),
                         _r(outbuf[:, 1, GRP - 1, :]),
                         start=False, stop=True, skip_group_check=True)
        y_sb = const.tile([1, B], F32)
        nc.scalar.activation(y_sb, py, AF.Sigmoid)
        nc.sync.dma_start(out=y[:], in_=y_sb)


def _prep_host(Ws, Us, bs, Wd, L, GRP, B_loc):
    """Pack all replicated weights into one [128, C] array (single DMA)."""
    Ws = np.asarray(Ws, np.float32)
    Us = np.asarray(Us, np.float32)
    bs = np.asarray(bs, np.float32)
    Wd = np.asarray(Wd, np.float32)
    has_bias = bool(np.any(bs != 0))
    has_bhh = bool(np.any(bs[:, :, 1, 2 * H:] != 0))
    CW = L * 2 * 2 * 3 * H
    CU = L * 2 * 3 * H
    GRPB = GRP * B_loc
    C = CW + CU + 2 + CU + L * H + GRPB + 4 * B_loc
    pack = np.zeros((H, C), np.float32)
    c_ones = CW + 2 * CU + 2 + L * H
    pack[0, c_ones:c_ones + GRPB] = 1.0           # ones row for bias MMs
    pack[0, c_ones + GRPB:c_ones + GRPB + B_loc] = 1.0        # ind2 row 0
    pack[1, c_ones + GRPB + B_loc:c_ones + GRPB + 2 * B_loc] = 1.0
    # w: [l, d, k(row-chunk), p(row within chunk), h] -> [p, (l d k h)]
    pack[:, :CW] = (Ws.reshape(L, 2, 2, H, 3 * H)
                    .transpose(3, 0, 1, 2, 4).reshape(H, CW))
    pack[:, CW:CW + CU] = (Us.transpose(2, 0, 1, 3).reshape(H, CU))
    pack[:, CW + CU] = Wd[0:H, 0]
    pack[:, CW + CU + 1] = Wd[H:2 * H, 0]
    if has_bias:
        bsum = bs[:, :, 0, :].copy()               # b_i everywhere
        bsum[:, :, :2 * H] += bs[:, :, 1, :2 * H]  # + b_h on z,r
        pack[0, CW + CU + 2:CW + 2 * CU + 2] = bsum.reshape(-1)
    if has_bhh:
        cb = CW + 2 * CU + 2
        pack[0:2, cb:cb + L * H] = np.transpose(
            bs[:, :, 1, 2 * H:], (1, 0, 2)).reshape(2, L * H)
    return {"wpack": pack}, has_bias, has_bhh


def run_gru(x, Ws, Us, bs, Wd, bd, n_cores=N_CORES, L=3, GRP=16, trace=False):
    x = np.ascontiguousarray(np.asarray(x, np.float32))
    B_full, T, _ = x.shape
    B_loc = B_full // n_cores
    common, has_bias, has_bhh = _prep_host(Ws, Us, bs, Wd, L, GRP, B_loc)

    nc = bacc.Bacc()
    build_gru(nc, B_loc, T, L, GRP, has_bias, has_bhh)
    nc.compile()

    in_maps = []
    for c in range(n_cores):
        m = dict(common)
        xs = x[c * B_loc:(c + 1) * B_loc]          # [B_loc, T, D]
        m["x"] = np.ascontiguousarray(
            xs.transpose(2, 1, 0).reshape(D_IN, T * B_loc))
        in_maps.append(m)

    res = run_bass_kernel_spmd(nc, in_maps, core_ids=list(range(n_cores)),
                               trace=trace)
    parts = [res.results[c]["y"][0] for c in range(n_cores)]
    out = np.concatenate(parts).reshape(B_full, 1).astype(np.float32)
    return out, res


def kernel(x, Ws, Us, bs, Wd, bd):
    bd = np.asarray(bd, np.float32).reshape(-1)
    out, _ = run_gru(x, Ws, Us, bs, Wd, bd)
    if np.any(bd != 0):
        # bd is zero in the spec; if not, fold it in via logit shift
        p = np.clip(np.float64(out), 1e-12, 1 - 1e-12)
        out = (1.0 / (1.0 + np.exp(-(np.log(p / (1 - p)) + bd[0]))))
    return np.asarray(out, np.float32)


# revision 20
# speedup vs baseline: 1.8639x; 1.8639x over previous
"""Trainium2 Bass kernel for a 3-layer bidirectional GRU + dense sigmoid head.

Problem: B=256, T=512, D=256, H=128 (Keras reset_after=True, gate order z,r,h).
Sharding: data-parallel over batch, 32 examples per core on 8 NeuronCores.

Per-core design (gate-partition layout, everything [128(h-dim), cols]):
- Input projections (xp = x @ W + b) are computed as chunked GEMMs whose
  outputs land directly in PSUM banks; the sequential scan's recurrence
  matmuls (h @ U) then accumulate on top of the same PSUM columns, so no
  PSUM->SBUF staging of xp is ever needed.
- Per 16-step group, PSUM banks: [z_f | r_f | z_b | r_b] (xp+rec, sigmoid
  reads all 4 with one strided AP), [xph_f | xph_b] (xp only), and a small
  per-step scratch bank for rec_h.
- Forward and backward chains are interleaved (independent recurrences) so
  the PE/ACT/DVE pipeline always has work.
- matmuls run in float32r (relaxed fp32), accumulation in fp32 PSUM.
"""

from contextlib import ExitStack

import numpy as np

import concourse.bass as bass
from concourse import bacc
import concourse.mybir as mybir
import concourse.tile as tile
from concourse.bass_utils import run_bass_kernel_spmd

H = 128
D_IN = 256
N_CORES = 8
F32 = mybir.dt.float32
F32R = mybir.dt.float32r
AF = mybir.ActivationFunctionType


def _r(ap):
    return ap.bitcast(F32R)


def build_gru(nc, B, T, L, GRP, has_bias, has_bhh):
    """Emit the full GRU program into `nc`."""
    NG = T // GRP
    assert T % GRP == 0

    # packed weights: cols [w | u | wd | bias(row0) | bhh(rows0-1)]
    CW = L * 2 * 2 * 3 * H          # 4608
    CU = L * 2 * 3 * H              # 2304
    c_u = CW
    c_wd = CW + CU
    c_bias = c_wd + 2
    c_bhh = c_bias + CU
    c_ones = c_bhh + L * H
    c_ind2 = c_ones + GRP * B
    c_h0 = c_ind2 + 2 * B
    C = c_h0 + 2 * B
    x = nc.dram_tensor("x", [D_IN, T * B], F32R, kind="ExternalInput")
    wpack = nc.dram_tensor("wpack", [H, C], F32R, kind="ExternalInput")
    y = nc.dram_tensor("y", [1, B], F32, kind="ExternalOutput")

    with tile.TileContext(nc) as tc, ExitStack() as ctx:
        const = ctx.enter_context(tc.tile_pool(name="const", bufs=1))
        rhsp = ctx.enter_context(tc.tile_pool(name="rhsp", bufs=2))
        outp = ctx.enter_context(tc.tile_pool(name="outp", bufs=2))
        stepp = ctx.enter_context(tc.tile_pool(name="stepp", bufs=3))
        psum = ctx.enter_context(tc.tile_pool(name="psum", bufs=1,
                                              space="PSUM"))
        pscr = ctx.enter_context(tc.tile_pool(name="pscr", bufs=2,
                                              space="PSUM"))
        dramp = ctx.enter_context(tc.tile_pool(name="dramp", bufs=1,
                                               space="DRAM"))

        # inter-layer hidden-sequence buffers (Tile-tracked DRAM)
        seqs = []
        for p in "AB":
            sf = dramp.tile([H, T * B], F32R, name=f"seq{p}f", tag=f"seq{p}f")
            sb = dramp.tile([H, T * B], F32R, name=f"seq{p}b", tag=f"seq{p}b")
            seqs.append((sf, sb))

        # ---- preload all weights with a single contiguous DMA ----
        pk = const.tile([H, C], F32R)
        nc.sync.dma_start(out=pk, in_=wpack[:])

        def w_ap(l, d, k, gi):
            c = ((l * 2 + d) * 2 + k) * 3 * H + gi * H
            return pk[:, c:c + H]

        def u_ap(l, d, gi):
            c = c_u + (l * 2 + d) * 3 * H + gi * H
            return pk[:, c:c + H]

        def wd_ap(d):
            return pk[:, c_wd + d:c_wd + d + 1]

        def bias_ap(l, d, gi):
            c = c_bias + (l * 2 + d) * 3 * H + gi * H
            return pk[0:1, c:c + H]

        def bhh_ap(l):
            return pk[0:2, c_bhh + l * H:c_bhh + (l + 1) * H]

        h0_sb = pk[:, c_h0:c_h0 + 2 * B].rearrange("p (d b) -> p d b", d=2)
        ones_sb = pk[0:1, c_ones:c_ones + GRP * B]
        ind2_sb = pk[0:2, c_ind2:c_ind2 + 2 * B]

        prev_out = None  # previous group's outbuf (h carry within a layer)
        outbuf = None

        def pair2(tile4, cf, cb):
            """[H, 2, B] view of a [H, 2, GRP, B] tile: fwd half at column
            cf, bwd half at column cb (asymmetric two-range AP)."""
            ps = tile4.ap[0][0]
            return bass.AP(tensor=tile4.tensor,
                           offset=tile4.offset + cf * B,
                           ap=[[ps, H], [(GRP + cb - cf) * B, 2], [1, B]])

        for l in range(L):
            for g in range(NG):
                # All DRAM traffic is t-ascending (contiguous DMA): the bwd
                # group g covers t in [T-GRP*(g+1), T-GRP*g) and the bwd
                # scan simply indexes its PSUM/SBUF columns in reverse.
                # ---- rhs tiles (moving operand of the xp GEMM) ----
                rhs = {}
                for d, dn in ((0, "f"), (1, "b")):
                    t_lo = GRP * g if d == 0 else T - GRP * (g + 1)
                    for k in range(2):
                        rt = rhsp.tile([H, GRP, B], F32R, tag=f"rhs{dn}{k}",
                                       name=f"rhs_{dn}{k}_{l}_{g}")
                        if l == 0:
                            s_fb = x[:][k * H:(k + 1) * H, :]
                        else:
                            s_fb = seqs[(l - 1) % 2][k]  # k0=fwd, k1=bwd half
                        src = s_fb.rearrange("p (t b) -> p t b", b=B)[
                            :, t_lo:t_lo + GRP, :]
                        nc.sync.dma_start(out=rt, in_=src)
                        rhs[(d, k)] = rt

                # ---- PSUM banks ----
                zrb = psum.tile([H, 4, GRP, B], F32, tag="zrb",
                                name=f"zrb_{l}_{g}")
                xph = psum.tile([H, 2, GRP, B], F32, tag="xph",
                                name=f"xph_{l}_{g}")

                # ---- xp GEMM: accumulate x @ W (+ b) into the banks ----
                for d in (0, 1):
                    for gi in range(3):
                        out_ap = (zrb[:, 2 * d + gi, :, :] if gi < 2
                                  else xph[:, d, :, :])
                        for k in range(2):
                            nc.tensor.matmul(
                                out_ap,
                                _r(w_ap(l, d, k, gi)),
                                _r(rhs[(d, k)]),
                                start=(k == 0), stop=False,
                                skip_group_check=True)
                        if has_bias:
                            nc.tensor.matmul(
                                out_ap,
                                _r(bias_ap(l, d, gi)),
                                _r(ones_sb),
                                start=False, stop=False,
                                skip_group_check=True)

                outbuf = outp.tile([H, 2, GRP, B], F32R, tag="outbuf",
                                   name=f"outbuf_{l}_{g}")

                # ---- the sequential scan: GRP fwd+bwd step-pairs ----
                # fwd step tl uses column tl; bwd step tl uses GRP-1-tl.
                for tl in range(GRP):
                    cb = GRP - 1 - tl
                    if prev_out is None and tl == 0:
                        hprev = h0_sb[:, :, :]  # [H, 2, B] zeros
                        hp_f, hp_b = h0_sb[:, 0, :], h0_sb[:, 1, :]
                    elif tl == 0:
                        hprev = pair2(prev_out, GRP - 1, 0)
                        hp_f = prev_out[:, 0, GRP - 1, :]
                        hp_b = prev_out[:, 1, 0, :]
                    else:
                        hprev = pair2(outbuf, tl - 1, cb + 1)
                        hp_f = outbuf[:, 0, tl - 1, :]
                        hp_b = outbuf[:, 1, cb + 1, :]

                    scratch = pscr.tile([H, 2, B], F32, tag="scratch",
                                        name=f"scr_{l}_{g}_{tl}")
                    for d, hp_d, col in ((0, hp_f, tl), (1, hp_b, cb)):
                        for gi in range(3):
                            out_ap = (zrb[:, 2 * d + gi, col, :] if gi < 2
                                      else scratch[:, d, :])
                            # h-gate writes the fresh scratch bank: start
                            # clears has_written for the WHOLE bank, so only
                            # the first direction may set it.
                            nc.tensor.matmul(
                                out_ap,
                                _r(u_ap(l, d, gi)),
                                _r(hp_d),
                                start=(gi == 2 and d == 0), stop=True,
                                skip_group_check=True)
                    if has_bhh:
                        nc.tensor.matmul(
                            scratch[:, :, :], _r(bhh_ap(l)),
                            _r(ind2_sb), start=False, stop=True,
                            skip_group_check=True)

                    # sigmoid over [z | r] per direction (different columns)
                    zrout = stepp.tile([H, 4, B], F32, tag="zrout",
                                       name=f"zrout_{l}_{g}_{tl}")
                    nc.scalar.activation(zrout[:, 0:2, :],
                                         zrb[:, 0:2, tl, :], AF.Sigmoid)
                    nc.scalar.activation(zrout[:, 2:4, :],
                                         zrb[:, 2:4, cb, :], AF.Sigmoid)
                    pstride = zrout.ap[0][0]
                    z_ap = bass.AP(tensor=zrout.tensor, offset=zrout.offset,
                                   ap=[[pstride, H], [2 * B, 2], [1, B]])
                    r_ap = bass.AP(tensor=zrout.tensor,
                                   offset=zrout.offset + B,
                                   ap=[[pstride, H], [2 * B, 2], [1, B]])

                    tt = stepp.tile([H, 2, B], F32, tag="tt",
                                    name=f"tt_{l}_{g}_{tl}")
                    nc.vector.tensor_mul(tt, scratch, r_ap)
                    arg = stepp.tile([H, 2, B], F32, tag="arg",
                                     name=f"arg_{l}_{g}_{tl}")
                    nc.vector.tensor_add(arg[:, 0, :], tt[:, 0, :],
                                         xph[:, 0, tl, :])
                    nc.vector.tensor_add(arg[:, 1, :], tt[:, 1, :],
                                         xph[:, 1, cb, :])
                    hh = stepp.tile([H, 2, B], F32, tag="hh",
                                    name=f"hh_{l}_{g}_{tl}")
                    nc.scalar.activation(hh, arg, AF.Tanh)
                    dd = stepp.tile([H, 2, B], F32, tag="dd",
                                    name=f"dd_{l}_{g}_{tl}")
                    nc.vector.tensor_sub(dd, hprev, hh)
                    ee = stepp.tile([H, 2, B], F32, tag="ee",
                                    name=f"ee_{l}_{g}_{tl}")
                    nc.vector.tensor_mul(ee, z_ap, dd)
                    nc.vector.tensor_add(pair2(outbuf, tl, cb), ee, hh)

                # ---- store the group's hidden states (layers 0..L-2) ----
                if l < L - 1:
                    sf, sb = seqs[l % 2]
                    nc.sync.dma_start(
                        out=sf.rearrange("p (t b) -> p t b", b=B)[
                            :, GRP * g:GRP * (g + 1), :],
                        in_=outbuf[:, 0, :, :])
                    t_lo_b = T - GRP * (g + 1)
                    nc.sync.dma_start(
                        out=sb.rearrange("p (t b) -> p t b", b=B)[
                            :, t_lo_b:t_lo_b + GRP, :],
                        in_=outbuf[:, 1, :, :])
                prev_out = outbuf
            prev_out = None  # h resets between layers

        # ---- dense head on the final states of the last group ----
        py = pscr.tile([1, B], F32, tag="scratch", name="py")
        nc.tensor.matmul(py, _r(wd_ap(0)),
                         _r(outbuf[:, 0, GRP - 1, :]),
                         start=True, stop=False, skip_group_check=True)
        nc.tensor.matmul(py, _r(wd_ap(1)),
                         _r(outbuf[:, 1, 0, :]),
                         start=False, stop=True, skip_group_check=True)
        y_sb = const.tile([1, B], F32)
        nc.scalar.activation(y_sb, py, AF.Sigmoid)
        nc.sync.dma_start(out=y[:], in_=y_sb)


def _prep_host(Ws, Us, bs, Wd, L, GRP, B_loc):
    """Pack all replicated weights into one [128, C] array (single DMA)."""
    Ws = np.asarray(Ws, np.float32)
    Us = np.asarray(Us, np.float32)
    bs = np.asarray(bs, np.float32)
    Wd = np.asarray(Wd, np.float32)
    has_bias = bool(np.any(bs != 0))
    has_bhh = bool(np.any(bs[:, :, 1, 2 * H:] != 0))
    CW = L * 2 * 2 * 3 * H
    CU = L * 2 * 3 * H
    GRPB = GRP * B_loc
    C = CW + CU + 2 + CU + L * H + GRPB + 4 * B_loc
    pack = np.zeros((H, C), np.float32)
    c_ones = CW + 2 * CU + 2 + L * H
    pack[0, c_ones:c_ones + GRPB] = 1.0           # ones row for bias MMs
    pack[0, c_ones + GRPB:c_ones + GRPB + B_loc] = 1.0        # ind2 row 0
    pack[1, c_ones + GRPB + B_loc:c_ones + GRPB + 2 * B_loc] = 1.0
    # w: [l, d, k(row-chunk), p(row within chunk), h] -> [p, (l d k h)]
    pack[:, :CW] = (Ws.reshape(L, 2, 2, H, 3 * H)
                    .transpose(3, 0, 1, 2, 4).reshape(H, CW))
    pack[:, CW:CW + CU] = (Us.transpose(2, 0, 1, 3).reshape(H, CU))
    pack[:, CW + CU] = Wd[0:H, 0]
    pack[:, CW + CU + 1] = Wd[H:2 * H, 0]
    if has_bias:
        bsum = bs[:, :, 0, :].copy()               # b_i everywhere
        bsum[:, :, :2 * H] += bs[:, :, 1, :2 * H]  # + b_h on z,r
        pack[0, CW + CU + 2:CW + 2 * CU + 2] = bsum.reshape(-1)
    if has_bhh:
        cb = CW + 2 * CU + 2
        pack[0:2, cb:cb + L * H] = np.transpose(
            bs[:, :, 1, 2 * H:], (1, 0, 2)).reshape(2, L * H)
    return {"wpack": pack}, has_bias, has_bhh


def run_gru(x, Ws, Us, bs, Wd, bd, n_cores=N_CORES, L=3, GRP=16, trace=False):
    x = np.ascontiguousarray(np.asarray(x, np.float32))
    B_full, T, _ = x.shape
    B_loc = B_full // n_cores
    common, has_bias, has_bhh = _prep_host(Ws, Us, bs, Wd, L, GRP, B_loc)

    nc = bacc.Bacc()
    build_gru(nc, B_loc, T, L, GRP, has_bias, has_bhh)
    nc.compile()

    in_maps = []
    for c in range(n_cores):
        m = dict(common)
        xs = x[c * B_loc:(c + 1) * B_loc]          # [B_loc, T, D]
        m["x"] = np.ascontiguousarray(
            xs.transpose(2, 1, 0).reshape(D_IN, T * B_loc))
        in_maps.append(m)

    res = run_bass_kernel_spmd(nc, in_maps, core_ids=list(range(n_cores)),
                               trace=trace)
    parts = [res.results[c]["y"][0] for c in range(n_cores)]
    out = np.concatenate(parts).reshape(B_full, 1).astype(np.float32)
    return out, res


def kernel(x, Ws, Us, bs, Wd, bd):
    bd = np.asarray(bd, np.float32).reshape(-1)
    out, _ = run_gru(x, Ws, Us, bs, Wd, bd)
    if np.any(bd != 0):
        # bd is zero in the spec; if not, fold it in via logit shift
        p = np.clip(np.float64(out), 1e-12, 1 - 1e-12)
        out = (1.0 / (1.0 + np.exp(-(np.log(p / (1 - p)) + bd[0]))))
    return np.asarray(out, np.float32)


# revision 21
# speedup vs baseline: 2.1247x; 1.1399x over previous
"""Trainium2 Bass kernel for a 3-layer bidirectional GRU + dense sigmoid head.

Problem: B=256, T=512, D=256, H=128 (Keras reset_after=True, gate order z,r,h).
Sharding: data-parallel over batch, 32 examples per core on 8 NeuronCores.

Per-core design (gate-partition layout, everything [128(h-dim), cols]):
- Input projections (xp = x @ W + b) are computed as chunked GEMMs whose
  outputs land directly in PSUM banks; the sequential scan's recurrence
  matmuls (h @ U) then accumulate on top of the same PSUM columns, so no
  PSUM->SBUF staging of xp is ever needed.
- Per 16-step group, PSUM banks: [z_f | r_f | z_b | r_b] (xp+rec, sigmoid
  reads all 4 with one strided AP), [xph_f | xph_b] (xp only), and a small
  per-step scratch bank for rec_h.
- Forward and backward chains are interleaved (independent recurrences) so
  the PE/ACT/DVE pipeline always has work.
- matmuls run in float32r (relaxed fp32), accumulation in fp32 PSUM.
"""

from contextlib import ExitStack

import numpy as np

import concourse.bass as bass
from concourse import bacc
import concourse.mybir as mybir
import concourse.tile as tile
from concourse.bass_utils import run_bass_kernel_spmd

H = 128
D_IN = 256
N_CORES = 8
F32 = mybir.dt.float32
F32R = mybir.dt.float32r
AF = mybir.ActivationFunctionType


def _r(ap):
    return ap.bitcast(F32R)


def build_gru(nc, B, T, L, GRP, has_bias, has_bhh):
    """Emit the full GRU program into `nc`."""
    NG = T // GRP
    assert T % GRP == 0

    # packed weights: cols [w | u | wd | bias(row0) | bhh(rows0-1)]
    CW = L * 2 * 2 * 3 * H          # 4608
    CU = L * 2 * 3 * H              # 2304
    c_u = CW
    c_wd = CW + CU
    c_bias = c_wd + 2
    c_bhh = c_bias + CU
    c_ones = c_bhh + L * H
    c_ind2 = c_ones + GRP * B
    c_h0 = c_ind2 + 2 * B
    C = c_h0 + 2 * B
    x = nc.dram_tensor("x", [D_IN, T * B], F32R, kind="ExternalInput")
    wpack = nc.dram_tensor("wpack", [H, C], F32R, kind="ExternalInput")
    y = nc.dram_tensor("y", [1, B], F32, kind="ExternalOutput")

    with tile.TileContext(nc) as tc, ExitStack() as ctx:
        const = ctx.enter_context(tc.tile_pool(name="const", bufs=1))
        rhsp = ctx.enter_context(tc.tile_pool(name="rhsp", bufs=2))
        outp = ctx.enter_context(tc.tile_pool(name="outp", bufs=2))
        stepp = ctx.enter_context(tc.tile_pool(name="stepp", bufs=3))
        psum = ctx.enter_context(tc.tile_pool(name="psum", bufs=1,
                                              space="PSUM"))
        pscr = ctx.enter_context(tc.tile_pool(name="pscr", bufs=2,
                                              space="PSUM"))
        dramp = ctx.enter_context(tc.tile_pool(name="dramp", bufs=1,
                                               space="DRAM"))

        # inter-layer hidden-sequence buffers (Tile-tracked DRAM)
        seqs = []
        for p in "AB":
            sf = dramp.tile([H, T * B], F32R, name=f"seq{p}f", tag=f"seq{p}f")
            sb = dramp.tile([H, T * B], F32R, name=f"seq{p}b", tag=f"seq{p}b")
            seqs.append((sf, sb))

        # ---- preload all weights with a single contiguous DMA ----
        pk = const.tile([H, C], F32R)
        nc.sync.dma_start(out=pk, in_=wpack[:])

        def w_ap(l, d, k, gi):
            c = ((l * 2 + d) * 2 + k) * 3 * H + gi * H
            return pk[:, c:c + H]

        def u_ap(l, d, gi):
            c = c_u + (l * 2 + d) * 3 * H + gi * H
            return pk[:, c:c + H]

        def wd_ap(d):
            return pk[:, c_wd + d:c_wd + d + 1]

        def bias_ap(l, d, gi):
            c = c_bias + (l * 2 + d) * 3 * H + gi * H
            return pk[0:1, c:c + H]

        def bhh_ap(l):
            return pk[0:2, c_bhh + l * H:c_bhh + (l + 1) * H]

        h0_sb = pk[:, c_h0:c_h0 + 2 * B].rearrange("p (d b) -> p d b", d=2)
        ones_sb = pk[0:1, c_ones:c_ones + GRP * B]
        ind2_sb = pk[0:2, c_ind2:c_ind2 + 2 * B]

        prev_out = None  # previous group's outbuf (h carry within a layer)
        outbuf = None

        def pair2(tile4, cf, cb):
            """[H, 2, B] view of a [H, 2, GRP, B] tile: fwd half at column
            cf, bwd half at column cb (asymmetric two-range AP)."""
            ps = tile4.ap[0][0]
            return bass.AP(tensor=tile4.tensor,
                           offset=tile4.offset + cf * B,
                           ap=[[ps, H], [(GRP + cb - cf) * B, 2], [1, B]])

        for l in range(L):
            for g in range(NG):
                # All DRAM traffic is t-ascending (contiguous DMA): the bwd
                # group g covers t in [T-GRP*(g+1), T-GRP*g) and the bwd
                # scan simply indexes its PSUM/SBUF columns in reverse.
                # ---- rhs tiles (moving operand of the xp GEMM) ----
                rhs = {}
                for d, dn in ((0, "f"), (1, "b")):
                    t_lo = GRP * g if d == 0 else T - GRP * (g + 1)
                    for k in range(2):
                        rt = rhsp.tile([H, GRP, B], F32R, tag=f"rhs{dn}{k}",
                                       name=f"rhs_{dn}{k}_{l}_{g}")
                        if l == 0:
                            s_fb = x[:][k * H:(k + 1) * H, :]
                        else:
                            s_fb = seqs[(l - 1) % 2][k]  # k0=fwd, k1=bwd half
                        src = s_fb.rearrange("p (t b) -> p t b", b=B)[
                            :, t_lo:t_lo + GRP, :]
                        nc.sync.dma_start(out=rt, in_=src)
                        rhs[(d, k)] = rt

                # ---- PSUM banks ----
                zrb = psum.tile([H, 4, GRP, B], F32, tag="zrb",
                                name=f"zrb_{l}_{g}")
                xph = psum.tile([H, 2, GRP, B], F32, tag="xph",
                                name=f"xph_{l}_{g}")

                # ---- xp GEMM: accumulate x @ W (+ b) into the banks ----
                for d in (0, 1):
                    for gi in range(3):
                        out_ap = (zrb[:, 2 * d + gi, :, :] if gi < 2
                                  else xph[:, d, :, :])
                        for k in range(2):
                            nc.tensor.matmul(
                                out_ap,
                                _r(w_ap(l, d, k, gi)),
                                _r(rhs[(d, k)]),
                                start=(k == 0), stop=False,
                                skip_group_check=True)
                        if has_bias:
                            nc.tensor.matmul(
                                out_ap,
                                _r(bias_ap(l, d, gi)),
                                _r(ones_sb),
                                start=False, stop=False,
                                skip_group_check=True)

                outbuf = outp.tile([H, 2, GRP, B], F32R, tag="outbuf",
                                   name=f"outbuf_{l}_{g}")

                # ---- the sequential scan: GRP fwd+bwd step-pairs ----
                # fwd step tl uses column tl; bwd step tl uses GRP-1-tl.
                for tl in range(GRP):
                    cb = GRP - 1 - tl
                    if prev_out is None and tl == 0:
                        hprev = h0_sb[:, :, :]  # [H, 2, B] zeros
                        hp_f, hp_b = h0_sb[:, 0, :], h0_sb[:, 1, :]
                    elif tl == 0:
                        hprev = pair2(prev_out, GRP - 1, 0)
                        hp_f = prev_out[:, 0, GRP - 1, :]
                        hp_b = prev_out[:, 1, 0, :]
                    else:
                        hprev = pair2(outbuf, tl - 1, cb + 1)
                        hp_f = outbuf[:, 0, tl - 1, :]
                        hp_b = outbuf[:, 1, cb + 1, :]

                    scratch = pscr.tile([H, 2, B], F32, tag="scratch",
                                        name=f"scr_{l}_{g}_{tl}")
                    for d, hp_d, col in ((0, hp_f, tl), (1, hp_b, cb)):
                        for gi in range(3):
                            out_ap = (zrb[:, 2 * d + gi, col, :] if gi < 2
                                      else scratch[:, d, :])
                            # h-gate writes the fresh scratch bank: start
                            # clears has_written for the WHOLE bank, so only
                            # the first direction may set it.
                            nc.tensor.matmul(
                                out_ap,
                                _r(u_ap(l, d, gi)),
                                _r(hp_d),
                                start=(gi == 2 and d == 0), stop=True,
                                skip_group_check=True)
                    if has_bhh:
                        nc.tensor.matmul(
                            scratch[:, :, :], _r(bhh_ap(l)),
                            _r(ind2_sb), start=False, stop=True,
                            skip_group_check=True)

                    # Per-direction gate math: fwd and bwd are independent
                    # recurrence chains; keeping their ops separate lets the
                    # engines pipeline one chain while the other waits on
                    # cross-engine semaphores.
                    zrout = stepp.tile([H, 4, B], F32, tag="zrout",
                                       name=f"zrout_{l}_{g}_{tl}")
                    tt = stepp.tile([H, 2, B], F32, tag="tt",
                                    name=f"tt_{l}_{g}_{tl}")
                    arg = stepp.tile([H, 2, B], F32, tag="arg",
                                     name=f"arg_{l}_{g}_{tl}")
                    hh = stepp.tile([H, 2, B], F32, tag="hh",
                                    name=f"hh_{l}_{g}_{tl}")
                    dd = stepp.tile([H, 2, B], F32, tag="dd",
                                    name=f"dd_{l}_{g}_{tl}")
                    ee = stepp.tile([H, 2, B], F32, tag="ee",
                                    name=f"ee_{l}_{g}_{tl}")
                    for d, hp_d, col in ((0, hp_f, tl), (1, hp_b, cb)):
                        zr_d = zrout[:, 2 * d:2 * d + 2, :]
                        nc.scalar.activation(zr_d, zrb[:, 2 * d:2 * d + 2,
                                                       col, :], AF.Sigmoid)
                        nc.vector.tensor_mul(tt[:, d, :], scratch[:, d, :],
                                             zrout[:, 2 * d + 1, :])
                        nc.vector.tensor_add(arg[:, d, :], tt[:, d, :],
                                             xph[:, d, col, :])
                        nc.scalar.activation(hh[:, d, :], arg[:, d, :],
                                             AF.Tanh)
                        nc.vector.tensor_sub(dd[:, d, :], hp_d, hh[:, d, :])
                        nc.vector.tensor_mul(ee[:, d, :],
                                             zrout[:, 2 * d, :], dd[:, d, :])
                        nc.vector.tensor_add(outbuf[:, d, col, :],
                                             ee[:, d, :], hh[:, d, :])

                # ---- store the group's hidden states (layers 0..L-2) ----
                if l < L - 1:
                    sf, sb = seqs[l % 2]
                    nc.sync.dma_start(
                        out=sf.rearrange("p (t b) -> p t b", b=B)[
                            :, GRP * g:GRP * (g + 1), :],
                        in_=outbuf[:, 0, :, :])
                    t_lo_b = T - GRP * (g + 1)
                    nc.sync.dma_start(
                        out=sb.rearrange("p (t b) -> p t b", b=B)[
                            :, t_lo_b:t_lo_b + GRP, :],
                        in_=outbuf[:, 1, :, :])
                prev_out = outbuf
            prev_out = None  # h resets between layers

        # ---- dense head on the final states of the last group ----
        py = pscr.tile([1, B], F32, tag="scratch", name="py")
        nc.tensor.matmul(py, _r(wd_ap(0)),
                         _r(outbuf[:, 0, GRP - 1, :]),
                         start=True, stop=False, skip_group_check=True)
        nc.tensor.matmul(py, _r(wd_ap(1)),
                         _r(outbuf[:, 1, 0, :]),
                         start=False, stop=True, skip_group_check=True)
        y_sb = const.tile([1, B], F32)
        nc.scalar.activation(y_sb, py, AF.Sigmoid)
        nc.sync.dma_start(out=y[:], in_=y_sb)


def _prep_host(Ws, Us, bs, Wd, L, GRP, B_loc):
    """Pack all replicated weights into one [128, C] array (single DMA)."""
    Ws = np.asarray(Ws, np.float32)
    Us = np.asarray(Us, np.float32)
    bs = np.asarray(bs, np.float32)
    Wd = np.asarray(Wd, np.float32)
    has_bias = bool(np.any(bs != 0))
    has_bhh = bool(np.any(bs[:, :, 1, 2 * H:] != 0))
    CW = L * 2 * 2 * 3 * H
    CU = L * 2 * 3 * H
    GRPB = GRP * B_loc
    C = CW + CU + 2 + CU + L * H + GRPB + 4 * B_loc
    pack = np.zeros((H, C), np.float32)
    c_ones = CW + 2 * CU + 2 + L * H
    pack[0, c_ones:c_ones + GRPB] = 1.0           # ones row for bias MMs
    pack[0, c_ones + GRPB:c_ones + GRPB + B_loc] = 1.0        # ind2 row 0
    pack[1, c_ones + GRPB + B_loc:c_ones + GRPB + 2 * B_loc] = 1.0
    # w: [l, d, k(row-chunk), p(row within chunk), h] -> [p, (l d k h)]
    pack[:, :CW] = (Ws.reshape(L, 2, 2, H, 3 * H)
                    .transpose(3, 0, 1, 2, 4).reshape(H, CW))
    pack[:, CW:CW + CU] = (Us.transpose(2, 0, 1, 3).reshape(H, CU))
    pack[:, CW + CU] = Wd[0:H, 0]
    pack[:, CW + CU + 1] = Wd[H:2 * H, 0]
    if has_bias:
        bsum = bs[:, :, 0, :].copy()               # b_i everywhere
        bsum[:, :, :2 * H] += bs[:, :, 1, :2 * H]  # + b_h on z,r
        pack[0, CW + CU + 2:CW + 2 * CU + 2] = bsum.reshape(-1)
    if has_bhh:
        cb = CW + 2 * CU + 2
        pack[0:2, cb:cb + L * H] = np.transpose(
            bs[:, :, 1, 2 * H:], (1, 0, 2)).reshape(2, L * H)
    return {"wpack": pack}, has_bias, has_bhh


def run_gru(x, Ws, Us, bs, Wd, bd, n_cores=N_CORES, L=3, GRP=16, trace=False):
    x = np.ascontiguousarray(np.asarray(x, np.float32))
    B_full, T, _ = x.shape
    B_loc = B_full // n_cores
    common, has_bias, has_bhh = _prep_host(Ws, Us, bs, Wd, L, GRP, B_loc)

    nc = bacc.Bacc()
    build_gru(nc, B_loc, T, L, GRP, has_bias, has_bhh)
    nc.compile()

    in_maps = []
    for c in range(n_cores):
        m = dict(common)
        xs = x[c * B_loc:(c + 1) * B_loc]          # [B_loc, T, D]
        m["x"] = np.ascontiguousarray(
            xs.transpose(2, 1, 0).reshape(D_IN, T * B_loc))
        in_maps.append(m)

    res = run_bass_kernel_spmd(nc, in_maps, core_ids=list(range(n_cores)),
                               trace=trace)
    parts = [res.results[c]["y"][0] for c in range(n_cores)]
    out = np.concatenate(parts).reshape(B_full, 1).astype(np.float32)
    return out, res


def kernel(x, Ws, Us, bs, Wd, bd):
    bd = np.asarray(bd, np.float32).reshape(-1)
    out, _ = run_gru(x, Ws, Us, bs, Wd, bd)
    if np.any(bd != 0):
        # bd is zero in the spec; if not, fold it in via logit shift
        p = np.clip(np.float64(out), 1e-12, 1 - 1e-12)
        out = (1.0 / (1.0 + np.exp(-(np.log(p / (1 - p)) + bd[0]))))
    return np.asarray(out, np.float32)
